# revision 47
# baseline (speedup 1.0000x reference)
"""AdaptiveRouter (MoE routing) Trainium2 kernel — 8-core data-parallel.

~99 us HW exec (neuron-profile, 8 cores SPMD); memory roofline ~57 us
(16.8 MB/core at the measured ~333 GB/s per-core HBM rate), TensorE busy
~73 us is the binding engine.

Design:
  - Token dim N=32768 sharded across 8 cores (4096 each); no collectives.
  - Host prep (layout only): per-core x shard transposed to [1024, 4096]
    and split into bf16 hi/lo halves (one stacked [2048, 4096] tensor), so
    the gate matmul runs as 3 bf16 matmuls (x_hi@W_hi + x_lo@W_hi +
    x_hi@W_lo) with near-fp32 accuracy (~5e-6 abs logit err). Gate weights
    padded to 128 stationary columns. TTHA weights packed into 3 blobs.
  - DMA: sync engine carries only the 16 1MB x-slab DMAs (full prefetch,
    ~333 GB/s); weight blobs ride gpsimd SWDGE, early-TTHA blob first.
  - TTHA adapter (batch-1) replicated per core in fp32, overlapping the x
    stream. Its chain avoids the busy Vector engine: PE does matmuls,
    transposes AND all bias adds (K=1 ones-row matmuls accumulated into
    the same PSUM region — one bias matmul per accumulation region, never
    spanning two); ACT applies activations straight to PSUM and does the
    LN reductions via accum_out; GpSimd does SBUF elementwise; inv_std =
    ACT Sqrt of DVE reciprocal (avoids ln/exp table-set thrash).
    tile_set_cur_wait hints align the scheduler's model with real chain
    timing so TTHA PE ops interleave into main-matmul gaps without
    head-of-line stalls.
  - Per 512-token group: 24 bf16 matmuls -> PSUM [(64)128, 512], raw
    logits to SBUF (DVE), 4 PE transposes into one PSUM bank, one copy to
    lbuf. Top-2 runs one group behind through a PE-broadcast [128, 64]
    bias tile: add + DVE Max8/MaxIndex, then a batched top-2 softmax
    epilogue (exact: w1 = 1/(1+e^(v2-v1)), w2 = e*w1).
  - Outputs are written in a HW-friendly [128, tiles*2] layout and
    de-interleaved on the host.
"""

import sys

sys.path.insert(0, "/opt/trn_rl_repo")

import numpy as np
import ml_dtypes

import concourse.bass as bass
import concourse.mybir as mybir
import concourse.tile as tile
from concourse import bacc
from concourse.bass_utils import run_bass_kernel_spmd
from concourse.masks import make_identity

F32 = mybir.dt.float32
BF16 = mybir.dt.bfloat16
I32 = mybir.dt.int32
U32 = mybir.dt.uint32
AF = mybir.ActivationFunctionType
OP = mybir.AluOpType
AX = mybir.AxisListType

N_CORES = 8
N, D, E, K = 32768, 1024, 64, 2
NT = N // N_CORES            # 4096 tokens per core
H = 256
G_TOK = 512                  # tokens per matmul group
N_GROUPS = NT // G_TOK       # 8
TILES = NT // 128            # 32 token tiles of 128
DC = D // 128                # 8 contraction chunks

BF = ml_dtypes.bfloat16

# ---- weight blobs (partition-major [128, *] f32) ----
# blob A: needed at the start of the TTHA chain
_WA_SEGS = [("catT", 4), ("cat_hw", 1), ("wc", 4 * 256), ("wh", 256)]
# blob B: needed a few microseconds later
_WB_SEGS = [
    ("wqkv", 2 * 768), ("wo", 2 * 256), ("wf", 2 * 256),
    ("wou1", 2 * 192), ("wo2", 64), ("wu2", 64),
]


def _offsets(segs):
    out, off = {}, 0
    for nm, w in segs:
        out[nm] = off
        off += w
    return out, off


_WA_OFF, WACOLS = _offsets(_WA_SEGS)
_WB_OFF, WBCOLS = _offsets(_WB_SEGS)

_B1_SEGS = [
    ("embb", 512), ("ln1g", 512), ("ln1b", 512), ("bf", 256), ("gf", 256),
    ("bef", 256), ("bo1", 128), ("bo2", 64), ("bu1", 64), ("bu2", 64),
    ("bgate", 64), ("bqkv", 768), ("bo", 256), ("bou1", 192),
]
_B1_OFF = {}
_off = 0
for _nm, _w in _B1_SEGS:
    _B1_OFF[_nm] = _off
    _off += _w
B1COLS = _off


def _act(nc, out, in_, func, scale=1.0, bias=0.0):
    nc.scalar.activation(out, in_, func, scale=scale, bias=bias)


def _layer_norm_gelu(nc, sb, ps_ap, g_ap, b_ap, out_ap, groups, width, tag):
    """out = gelu(LN(ps) * g + b) on one partition; LN per group of `width`.

    Chain uses ONLY ACT (reductions via accum_out, per-group scalars as
    bias/scale APs) + GpSimd scalar fixups + the gelu — the Vector engine
    stays free for main-loop throughput work.
    """
    AFT = AF
    scratch = sb.tile([1, groups, width], F32, tag=f"{tag}_scr")
    xc = sb.tile([1, groups, width], F32, tag=f"{tag}_xc")
    xn = sb.tile([1, groups, width], F32, tag=f"{tag}_xn")
    mu = sb.tile([1, groups, 1], F32, tag=f"{tag}_mu")
    ssq = sb.tile([1, groups, 1], F32, tag=f"{tag}_ssq")
    g3 = g_ap.rearrange("o (g w) -> o g w", g=groups)
    b3 = b_ap.rearrange("o (g w) -> o g w", g=groups)
    o3 = out_ap.rearrange("o (g w) -> o g w", g=groups)
    x3 = ps_ap.rearrange("o (g w) -> o g w", g=groups)

    for gi in range(groups):
        nc.scalar.activation(scratch[:, gi, :], x3[:, gi, :], AFT.Copy,
                             accum_out=mu[:, gi, :])
    # neg_mu = -sum/width
    nc.gpsimd.tensor_scalar(mu[:], mu[:], -1.0 / width, None, op0=OP.mult)
    for gi in range(groups):
        nc.scalar.activation(xc[:, gi, :], x3[:, gi, :], AFT.Identity,
                             bias=mu[:, gi, :])
        nc.scalar.activation(scratch[:, gi, :], xc[:, gi, :], AFT.Square,
                             accum_out=ssq[:, gi, :])
    # inv_std = sqrt(1/(ssq/width + eps))
    nc.gpsimd.tensor_scalar(ssq[:], ssq[:], 1.0 / width, 1e-5, op0=OP.mult,
                            op1=OP.add)
    nc.vector.reciprocal(ssq[:, :, 0], ssq[:, :, 0])
    nc.scalar.activation(ssq[:, :, 0], ssq[:, :, 0], AFT.Sqrt)
    for gi in range(groups):
        nc.scalar.activation(xn[:, gi, :], xc[:, gi, :], AFT.Identity,
                             scale=ssq[:, gi, :])
    nc.gpsimd.tensor_tensor(xn[:], xn[:], g3, op=OP.mult)
    nc.gpsimd.tensor_tensor(xn[:], xn[:], b3, op=OP.add)
    nc.scalar.activation(o3, xn[:], AFT.Gelu)


def build():
    nc = bacc.Bacc(target_bir_lowering=False)

    xz = nc.dram_tensor("xz", [2 * D, NT], BF16, kind="ExternalInput")
    wgz = nc.dram_tensor("wgz", [2 * D, 128], BF16, kind="ExternalInput")
    wbloba = nc.dram_tensor("wbloba", [128, WACOLS], F32, kind="ExternalInput")
    wblobb = nc.dram_tensor("wblobb", [128, WBCOLS], F32, kind="ExternalInput")
    bblob1 = nc.dram_tensor("bblob1", [1, B1COLS], F32, kind="ExternalInput")

    out_w = nc.dram_tensor("out_w", [128, TILES * 2], F32, kind="ExternalOutput")
    out_i = nc.dram_tensor("out_i", [128, TILES * 2], I32, kind="ExternalOutput")
    out_rb = nc.dram_tensor("out_rb", [1, E], F32, kind="ExternalOutput")
    out_un = nc.dram_tensor("out_un", [1, E], F32, kind="ExternalOutput")

    with tile.TileContext(nc) as tc:
        with tc.tile_pool(name="const", bufs=1) as cs, \
             tc.tile_pool(name="tt", bufs=1) as ts, \
             tc.tile_pool(name="xs", bufs=8) as xs, \
             tc.tile_pool(name="wk", bufs=3) as wk, \
             tc.tile_pool(name="pmain", bufs=4, space="PSUM") as pmain, \
             tc.tile_pool(name="ptr", bufs=2, space="PSUM") as ptr, \
             tc.tile_pool(name="ptt", bufs=2, space="PSUM") as ptt:

            # ---- DMA plan: the sync engine carries ONLY the 16 x-slab halves
            # (the bandwidth-critical stream); all weights/biases ride the
            # gpsimd SWDGE queue, with the early-TTHA blob first.
            wz_s = cs.tile([128, 2 * DC, 128], BF16)
            nc.gpsimd.dma_start(wz_s[:], wgz[:].rearrange("(c p) e -> p c e", p=128))
            wba = cs.tile([128, WACOLS], F32)
            nc.gpsimd.dma_start(wba[:], wbloba[:])
            b1 = cs.tile([1, B1COLS], F32)
            nc.gpsimd.dma_start(b1[:], bblob1[:])
            wbb = cs.tile([128, WBCOLS], F32)
            nc.gpsimd.dma_start(wbb[:], wblobb[:])

            slabs = []
            for g in range(N_GROUPS):
                xhi_g = xs.tile([128, DC, G_TOK], BF16, tag="xhi")
                xlo_g = xs.tile([128, DC, G_TOK], BF16, tag="xlo")
                nc.sync.dma_start(
                    xhi_g[:], xz[0:D, g * G_TOK:(g + 1) * G_TOK].rearrange(
                        "(c p) t -> p c t", p=128))
                nc.sync.dma_start(
                    xlo_g[:], xz[D:2 * D, g * G_TOK:(g + 1) * G_TOK].rearrange(
                        "(c p) t -> p c t", p=128))
                slabs.append((xhi_g, xlo_g))

            ident = cs.tile([128, 128], F32)
            make_identity(nc, ident[:])

            def wseg(nm, rows=128):
                if nm in _WA_OFF:
                    off = _WA_OFF[nm]
                    w = dict(_WA_SEGS)[nm]
                    return wba[0:rows, off:off + w]
                off = _WB_OFF[nm]
                w = dict(_WB_SEGS)[nm]
                return wbb[0:rows, off:off + w]

            def b1seg(nm, w):
                off = _B1_OFF[nm]
                return b1[:, off:off + w]

            # persistent result buffers
            vbuf8 = cs.tile([128, TILES * 8], F32)
            ibuf8 = cs.tile([128, TILES * 8], U32)
            wbuf = cs.tile([128, TILES, 2], F32)
            obuf = cs.tile([128, TILES, 2], I32)

            # =========================================================
            # TTHA adapter (fp32, replicated per core).
            # Engine plan: PE does matmuls/transposes AND all bias adds
            # (as K=1 ones-row matmuls accumulated into the same PSUM);
            # ACT applies activations/copies directly to PSUM; GpSimd does
            # SBUF elementwise; DVE only the reductions + reciprocal.
            # =========================================================
            _hp = tc.high_priority()
            _hp.__enter__()
            ones1 = ts.tile([3, 128], F32)
            nc.gpsimd.memset(ones1[:], 1.0)

            def bias_mm(ps_ap, rows, nm, width, off=0):
                # ps[r, :] += bias_row  for all r in rows
                nc.tensor.matmul(ps_ap, ones1[0:1, 0:rows],
                                 b1[:, _B1_OFF[nm] + off:_B1_OFF[nm] + off + width],
                                 start=False, stop=True, skip_group_check=True)

            catT = wseg("catT").rearrange("p (c o) -> p c o", c=4)
            wc = wseg("wc").rearrange("p (c n) -> p c n", c=4)
            ps_emb = ptt.tile([1, 2 * H], F32, tag="ptt")
            for c in range(4):
                nc.tensor.matmul(ps_emb[:, 0:H], catT[:, c, :], wc[:, c, :],
                                 start=(c == 0), stop=(c == 3))
            nc.tensor.matmul(ps_emb[:, H:2 * H], wseg("cat_hw", rows=9),
                             wseg("wh", rows=9), start=True, stop=True)
            emb_act = ts.tile([1, 2 * H], F32)
            _layer_norm_gelu(nc, ts, ps_emb[:], b1seg("ln1g", 512), b1seg("ln1b", 512),
                             emb_act[:], groups=2, width=H, tag="ln1")
            # combT [128, 2, 3]
            tc.tile_set_cur_wait(0.028)
            combT = ts.tile([128, 2, 3], F32)
            nc.gpsimd.memset(combT[:], 0.0)
            for c in range(2):
                ps_cb = ptt.tile([128, 2], F32, tag="ptt")
                for j in range(2):
                    nc.tensor.transpose(ps_cb[:, j:j + 1],
                                        emb_act[0:1, (j * H + c * 128):(j * H + c * 128 + 128)],
                                        ident[0:1, 0:1])
                nc.scalar.copy(combT[:, c, 0:2], ps_cb[:])

            # qkv = comb @ Wqkv + bqkv -> [3, 768]
            wqkv = wseg("wqkv").rearrange("p (c n) -> p c n", c=2)
            qkv_s = ts.tile([3, 3 * H], F32)
            for half in range(2):
                ps_q = ptt.tile([3, 384], F32, tag="ptt")
                for c in range(2):
                    nc.tensor.matmul(ps_q[:], combT[:, c, :],
                                     wqkv[:, c, (half * 384):(half * 384 + 384)],
                                     start=(c == 0), stop=False, skip_group_check=True)
                bias_mm(ps_q[:], 3, "bqkv", 384, off=half * 384)
                nc.scalar.copy(qkv_s[:, half * 384:(half + 1) * 384], ps_q[:])

            # qT / kT packed 3 heads per chunk (score scale folded into Exp)
            qT = ts.tile([128, 3, 3], F32)
            kT = ts.tile([128, 3, 3], F32)
            for c in range(3):
                w = 96 if c < 2 else 64
                ps_t = ptt.tile([128, 3], F32, tag="ptt")
                nc.tensor.transpose(ps_t[:w], qkv_s[0:3, c * 96:(c * 96 + w)],
                                    ident[0:3, 0:3])
                nc.scalar.copy(qT[:w, c, :], ps_t[:w])
                ps_t2 = ptt.tile([128, 3], F32, tag="ptt")
                nc.tensor.transpose(ps_t2[:w], qkv_s[0:3, (H + c * 96):(H + c * 96 + w)],
                                    ident[0:3, 0:3])
                nc.scalar.copy(kT[:w, c, :], ps_t2[:w])

            tc.tile_set_cur_wait(0.035)
            ps_sc = ptt.tile([3, 24], F32, tag="ptt")
            for h in range(8):
                c, hh = divmod(h, 3)
                nc.tensor.matmul(ps_sc[:, h * 3:(h + 1) * 3],
                                 qT[hh * 32:(hh + 1) * 32, c, :],
                                 kT[hh * 32:(hh + 1) * 32, c, :],
                                 start=True, stop=True)
            # softmax over j: exp straight off PSUM (scale folded in), heads
            # packed 3-per-96 so THREE transposes cover all 8; row sums on
            # DVE, reciprocal as exp(-ln), normalization on GpSimd.
            s32 = ts.tile([3, 3, 3, 32], F32)     # (chunk, head-in-chunk, j-pad)
            nc.gpsimd.memset(s32[:], 0.0)
            _act(nc, s32[:, 0:2, :, 0:3],
                 ps_sc[:, 0:18].rearrange("p (c s j) -> p c s j", c=2, s=3),
                 AF.Exp, scale=float(1.0 / np.sqrt(32.0)))
            _act(nc, s32[:, 2:3, 0:2, 0:3],
                 ps_sc[:, 18:24].rearrange("p (c s j) -> p c s j", c=1, s=2),
                 AF.Exp, scale=float(1.0 / np.sqrt(32.0)))
            at_s = ts.tile([3, 8, 3], F32)
            ps_at = ptt.tile([3, 24], F32, tag="ptt")
            s32v = s32[:].rearrange("p c s o -> p (c s) o")     # [3, 9, 32]
            for h in range(8):
                c, hh = divmod(h, 3)
                nc.tensor.transpose(ps_at[:, h * 3:(h + 1) * 3],
                                    s32v[0:3, c * 3 + hh, 0:3], ident[0:3, 0:3])
            nc.scalar.copy(at_s[:], ps_at[:].rearrange("p (h j) -> p h j", h=8))
            ps_rs = ptt.tile([3, 8], F32, tag="ptt")
            for h in range(8):
                nc.tensor.matmul(ps_rs[:, h:h + 1], at_s[:, h, :], ones1[0:3, 0:1],
                                 start=True, stop=True)
            rinv = ts.tile([3, 8], F32)
            nc.vector.reciprocal(rinv[:], ps_rs[:])

            ps_ctx = ptt.tile([3, H], F32, tag="ptt")
            for h in range(8):
                nc.tensor.matmul(ps_ctx[:, h * 32:(h + 1) * 32], at_s[:, h, :],
                                 qkv_s[0:3, (2 * H + h * 32):(2 * H + (h + 1) * 32)],
                                 start=True, stop=True)
            ctx_s = ts.tile([3, 8, 32], F32)
            nc.scalar.copy(ctx_s[:], ps_ctx[:].rearrange("p (h d) -> p h d", h=8))
            nc.gpsimd.tensor_tensor(ctx_s[:], ctx_s[:],
                                    rinv[:, :, None].to_broadcast([3, 8, 32]),
                                    op=OP.mult)
            ctx_f = ctx_s[:].rearrange("p h d -> p (h d)")

            ctxT = ts.tile([128, 2, 3], F32)
            for c in range(2):
                ps_ct = ptt.tile([128, 3], F32, tag="ptt")
                nc.tensor.transpose(ps_ct[:], ctx_f[0:3, c * 128:(c + 1) * 128],
                                    ident[0:3, 0:3])
                nc.scalar.copy(ctxT[:, c, :], ps_ct[:])
            wo = wseg("wo").rearrange("p (c n) -> p c n", c=2)
            ps_att = ptt.tile([3, H], F32, tag="ptt")
            for c in range(2):
                nc.tensor.matmul(ps_att[:], ctxT[:, c, :], wo[:, c, :],
                                 start=(c == 0), stop=False, skip_group_check=True)
            bias_mm(ps_att[:], 3, "bo", H)
            att_s = ts.tile([3, H], F32)
            nc.scalar.copy(att_s[:], ps_att[:])

            tc.tile_set_cur_wait(0.044)
            third = ts.tile([3, 1], F32)
            nc.gpsimd.memset(third[:], 1.0 / 3.0)
            ps_mean = ptt.tile([1, H], F32, tag="ptt")
            nc.tensor.matmul(ps_mean[:], third[:], att_s[:], start=True, stop=True)
            mean_s = ts.tile([1, H], F32)
            nc.scalar.copy(mean_s[:], ps_mean[:])

            meanT = ts.tile([128, 2, 1], F32)
            for c in range(2):
                ps_mt = ptt.tile([128, 1], F32, tag="ptt")
                nc.tensor.transpose(ps_mt[:], mean_s[0:1, c * 128:(c + 1) * 128],
                                    ident[0:1, 0:1])
                nc.scalar.copy(meanT[:, c, :], ps_mt[:])
            wf = wseg("wf").rearrange("p (c n) -> p c n", c=2)
            ps_f = ptt.tile([1, H], F32, tag="ptt")
            for c in range(2):
                nc.tensor.matmul(ps_f[:], meanT[:, c, :], wf[:, c, :],
                                 start=(c == 0), stop=False, skip_group_check=True)
            bias_mm(ps_f[:], 1, "bf", H)
            fused = ts.tile([1, H], F32)
            _layer_norm_gelu(nc, ts, ps_f[:], b1seg("gf", H), b1seg("bef", H),
                             fused[:], groups=1, width=H, tag="ln2")

            tc.tile_set_cur_wait(0.050)
            fusedT = ts.tile([128, 2, 1], F32)
            for c in range(2):
                ps_ft = ptt.tile([128, 1], F32, tag="ptt")
                nc.tensor.transpose(ps_ft[:], fused[0:1, c * 128:(c + 1) * 128],
                                    ident[0:1, 0:1])
                nc.scalar.copy(fusedT[:, c, :], ps_ft[:])

            # heads (Wo1|Wu1 merged into one [256,192] matmul)
            wou1 = wseg("wou1").rearrange("p (c n) -> p c n", c=2)
            ps_hu = ptt.tile([1, 192], F32, tag="ptt")
            for c in range(2):
                nc.tensor.matmul(ps_hu[:], fusedT[:, c, :], wou1[:, c, :],
                                 start=(c == 0), stop=False, skip_group_check=True)
            bias_mm(ps_hu[:], 1, "bou1", 192)
            hu = ts.tile([1, 192], F32)
            _act(nc, hu[:], ps_hu[:], AF.Gelu)
            h1 = hu[:, 0:128]
            u1 = hu[:, 128:192]

            h1T = ts.tile([128, 1], F32)
            ps_h1t = ptt.tile([128, 1], F32, tag="ptt")
            nc.tensor.transpose(ps_h1t[:], h1, ident[0:1, 0:1])
            nc.scalar.copy(h1T[:], ps_h1t[:])
            ps_rb = ptt.tile([1, E], F32, tag="ptt")
            nc.tensor.matmul(ps_rb[:], h1T[:], wseg("wo2"), start=True, stop=False,
                             skip_group_check=True)
            bias_mm(ps_rb[:], 1, "bo2", E)
            rb = ts.tile([1, E], F32)
            _act(nc, rb[:], ps_rb[:], AF.Tanh)  # tanh lives in the gelu set
            nc.gpsimd.dma_start(out_rb[:], rb[:])

            u1T = ts.tile([64, 1], F32)
            ps_u1t = ptt.tile([64, 1], F32, tag="ptt")
            nc.tensor.transpose(ps_u1t[:], u1, ident[0:1, 0:1])
            nc.scalar.copy(u1T[:], ps_u1t[:])
            ps_u2 = ptt.tile([1, E], F32, tag="ptt")
            nc.tensor.matmul(ps_u2[:], u1T[:], wseg("wu2", rows=64), start=True,
                             stop=False, skip_group_check=True)
            bias_mm(ps_u2[:], 1, "bu2", E)
            un = ts.tile([1, E], F32)
            _act(nc, un[:], ps_u2[:], AF.Exp)
            nc.gpsimd.tensor_scalar(un[:], un[:], 1.0, None, op0=OP.add)
            _act(nc, un[:], un[:], AF.Ln)
            nc.gpsimd.dma_start(out_un[:], un[:])

            # total per-expert bias broadcast to all 128 partitions via PE
            bt = ts.tile([1, E], F32)
            nc.gpsimd.tensor_tensor(bt[:], rb[:], b1seg("bgate", E), op=OP.add)
            ps_b128 = ptt.tile([128, E], F32, tag="ptt")
            nc.tensor.matmul(ps_b128[:], ones1[0:1, :], bt[:], start=True, stop=True)
            bias128 = cs.tile([128, E], F32)
            nc.scalar.copy(bias128[:], ps_b128[:])
            tc.cur_wait_ts = None
            _hp.__exit__(None, None, None)

            # =========================================================
            # Main gate matmul + top-2.  The PE pipeline (matmuls +
            # transposes) never waits on the TTHA chain: raw logits are
            # transposed into lbuf, and the bias add + top-2 for group g
            # run one group behind (by which time bias128 is ready).
            # =========================================================
            lbuf = cs.tile([128, TILES, E], F32)

            def gated_top2(g):
                tmp4 = wk.tile([128, 4, E], F32, tag="tmp4")
                nc.vector.tensor_tensor(
                    tmp4[:], lbuf[:, g * 4:(g + 1) * 4, :],
                    bias128[:, None, :].to_broadcast([128, 4, E]), op=OP.add)
                for t in range(4):
                    tl = g * 4 + t
                    nc.vector.max(vbuf8[:, tl * 8:(tl + 1) * 8], tmp4[:, t, :])
                    nc.vector.max_index(ibuf8[:, tl * 8:(tl + 1) * 8],
                                        vbuf8[:, tl * 8:(tl + 1) * 8], tmp4[:, t, :])

            for g in range(N_GROUPS):
                xhi_g, xlo_g = slabs[g]
                ps_lg = pmain.tile([128, G_TOK], F32, tag="lg")
                n_mm = 3 * DC
                k = 0
                # hi products first (only need the hi half of the slab)
                for c in range(DC):
                    nc.tensor.matmul(ps_lg[:], wz_s[:, c, :], xhi_g[:, c, :],
                                     start=(k == 0), stop=False)
                    k += 1
                for c in range(DC):
                    nc.tensor.matmul(ps_lg[:], wz_s[:, c, :], xlo_g[:, c, :],
                                     start=False, stop=False)
                    k += 1
                    nc.tensor.matmul(ps_lg[:], wz_s[:, DC + c, :], xhi_g[:, c, :],
                                     start=False, stop=(k == n_mm - 1))
                    k += 1

                # raw (unbiased) logits to SBUF (DVE; ACT stays pure-chain)
                adj = wk.tile([E, G_TOK], F32, tag="adj")
                nc.vector.tensor_copy(adj[:], ps_lg[0:E, :])

                ps_tr = ptr.tile([128, 4, E], F32, tag="tr")
                for t in range(4):
                    nc.tensor.transpose(ps_tr[:, t, :], adj[:, t * 128:(t + 1) * 128],
                                        ident[0:E, 0:E])
                nc.vector.tensor_copy(lbuf[:, g * 4:(g + 1) * 4, :], ps_tr[:])

                if g >= 1:
                    gated_top2(g - 1)
            gated_top2(N_GROUPS - 1)

            # ---- top-2 softmax epilogue ----
            v3 = vbuf8[:].rearrange("p (t k) -> p t k", k=8)
            i3 = ibuf8[:].rearrange("p (t k) -> p t k", k=8)
            d_t = wk.tile([128, TILES, 1], F32, tag="dt")
            nc.vector.tensor_tensor(d_t[:], v3[:, :, 1:2], v3[:, :, 0:1], op=OP.subtract)
            _act(nc, d_t[:], d_t[:], AF.Exp)
            s_t = wk.tile([128, TILES, 1], F32, tag="st")
            nc.vector.tensor_scalar(s_t[:], d_t[:], 1.0, None, op0=OP.add)
            nc.vector.reciprocal(wbuf[:, :, 0:1], s_t[:])
            nc.vector.tensor_tensor(wbuf[:, :, 1:2], d_t[:], wbuf[:, :, 0:1], op=OP.mult)
            nc.vector.tensor_copy(obuf[:, :, 0:1], i3[:, :, 0:1])
            nc.vector.tensor_copy(obuf[:, :, 1:2], i3[:, :, 1:2])
            nc.sync.dma_start(out_w[:], wbuf[:])
            nc.sync.dma_start(out_i[:], obuf[:])

    nc.finalize()
    return nc


_CACHE = {}


def _get_nc():
    if "nc" not in _CACHE:
        _CACHE["nc"] = build()
    return _CACHE["nc"]


def kernel(**inputs):
    f32 = np.float32
    g = {k: np.asarray(v, f32) for k, v in inputs.items()}
    x = g["x"]

    wg = g["W_gate"]
    wghi = wg.astype(BF)
    wglo = (wg - wghi.astype(f32)).astype(BF)
    wgz = np.zeros((2 * D, 128), BF)
    wgz[:, 0:E] = np.concatenate([wghi, wglo], axis=0)

    wbloba = np.zeros((128, WACOLS), f32)
    wblobb = np.zeros((128, WBCOLS), f32)

    def put_w(nm, arr):
        blob, off = ((wbloba, _WA_OFF[nm]) if nm in _WA_OFF
                     else (wblobb, _WB_OFF[nm]))
        arr = np.asarray(arr, f32)
        blob[:arr.shape[0], off:off + arr.shape[1]] = arr

    cost = g["cost_features"][0]
    catT_arr = np.zeros((128, 4), f32)
    catT_arr[:, 0:3] = cost.reshape(3, 128).T
    catT_arr[0, 3] = 1.0
    put_w("catT", catT_arr)
    cat_hw = np.zeros((9, 1), f32)
    cat_hw[0:8, 0] = g["hardware_features"][0]
    cat_hw[8, 0] = 1.0
    put_w("cat_hw", cat_hw)
    wc_arr = np.zeros((128, 4, 256), f32)
    wc_arr[:, 0:3, :] = g["Wc"].reshape(3, 128, 256).transpose(1, 0, 2)
    wc_arr[0, 3, :] = g["bc"]
    put_w("wc", wc_arr.reshape(128, 4 * 256))
    wh_arr = np.zeros((9, 256), f32)
    wh_arr[0:8] = g["Wh"]
    wh_arr[8] = g["bh"]
    put_w("wh", wh_arr)
    put_w("wqkv", g["Wqkv"].reshape(2, 128, 768).transpose(1, 0, 2).reshape(128, 2 * 768))
    put_w("wo", g["Wo"].reshape(2, 128, 256).transpose(1, 0, 2).reshape(128, 2 * 256))
    put_w("wf", g["Wf"].reshape(2, 128, 256).transpose(1, 0, 2).reshape(128, 2 * 256))
    wou1 = np.concatenate([g["Wo1"], g["Wu1"]], axis=1)
    put_w("wou1", wou1.reshape(2, 128, 192).transpose(1, 0, 2).reshape(128, 2 * 192))
    put_w("wo2", g["Wo2"])
    put_w("wu2", g["Wu2"])

    bblob1 = np.zeros((1, B1COLS), f32)

    def put_b(nm, arr):
        off = _B1_OFF[nm]
        arr = np.asarray(arr, f32).reshape(-1)
        bblob1[0, off:off + arr.size] = arr

    put_b("embb", np.concatenate([g["bc"], g["bh"]]))
    put_b("ln1g", np.concatenate([g["gc"], g["gh"]]))
    put_b("ln1b", np.concatenate([g["bec"], g["beh"]]))
    put_b("bf", g["bf"]); put_b("gf", g["gf"]); put_b("bef", g["bef"])
    put_b("bo1", g["bo1"]); put_b("bo2", g["bo2"])
    put_b("bu1", g["bu1"]); put_b("bu2", g["bu2"])
    put_b("bou1", np.concatenate([g["bo1"], g["bu1"]]))
    put_b("bgate", g["b_gate"])

    put_b("bqkv", g["bqkv"])
    put_b("bo", g["bo"])

    shared = dict(wgz=wgz, wbloba=wbloba, wblobb=wblobb, bblob1=bblob1)

    in_maps = []
    for c in range(N_CORES):
        xs = np.ascontiguousarray(x[c * NT:(c + 1) * NT].T)
        xhi = xs.astype(BF)
        xlo = (xs - xhi.astype(f32)).astype(BF)
        in_maps.append(dict(shared, xz=np.concatenate([xhi, xlo], axis=0)))

    nc = _get_nc()
    res = run_bass_kernel_spmd(nc, in_maps, core_ids=list(range(N_CORES)))

    weights = np.empty((N, K), f32)
    top_idx = np.empty((N, K), np.int32)
    for c in range(N_CORES):
        r = res.results[c]
        weights[c * NT:(c + 1) * NT] = (
            r["out_w"].reshape(128, TILES, 2).transpose(1, 0, 2).reshape(NT, 2))
        top_idx[c * NT:(c + 1) * NT] = (
            r["out_i"].reshape(128, TILES, 2).transpose(1, 0, 2).reshape(NT, 2))
    rb = res.results[0]["out_rb"].reshape(1, E).astype(f32)
    un = res.results[0]["out_un"].reshape(1, E).astype(f32)
    return weights, top_idx, rb, un


# revision 48
# speedup vs baseline: 1.1925x; 1.1925x over previous
"""AdaptiveRouter (MoE routing) Trainium2 kernel — 8-core data-parallel.

~99 us HW exec (neuron-profile, 8 cores SPMD); memory roofline ~57 us
(16.8 MB/core at the measured ~333 GB/s per-core HBM rate), TensorE busy
~73 us is the binding engine.

Design:
  - Token dim N=32768 sharded across 8 cores (4096 each); no collectives.
  - Host prep (layout only): per-core x shard transposed to [1024, 4096]
    and split into bf16 hi/lo halves (one stacked [2048, 4096] tensor), so
    the gate matmul runs as 3 bf16 matmuls (x_hi@W_hi + x_lo@W_hi +
    x_hi@W_lo) with near-fp32 accuracy (~5e-6 abs logit err). Gate weights
    padded to 128 stationary columns. TTHA weights packed into 3 blobs.
  - DMA: sync engine carries only the 16 1MB x-slab DMAs (full prefetch,
    ~333 GB/s); weight blobs ride gpsimd SWDGE, early-TTHA blob first.
  - TTHA adapter (batch-1) replicated per core in fp32, overlapping the x
    stream. Its chain avoids the busy Vector engine: PE does matmuls,
    transposes AND all bias adds (K=1 ones-row matmuls accumulated into
    the same PSUM region — one bias matmul per accumulation region, never
    spanning two); ACT applies activations straight to PSUM and does the
    LN reductions via accum_out; GpSimd does SBUF elementwise; inv_std =
    ACT Sqrt of DVE reciprocal (avoids ln/exp table-set thrash).
    tile_set_cur_wait hints align the scheduler's model with real chain
    timing so TTHA PE ops interleave into main-matmul gaps without
    head-of-line stalls.
  - Per 512-token group: 24 bf16 matmuls -> PSUM [(64)128, 512], raw
    logits to SBUF (DVE), 4 PE transposes into one PSUM bank, one copy to
    lbuf. Top-2 runs one group behind through a PE-broadcast [128, 64]
    bias tile: add + DVE Max8/MaxIndex, then a batched top-2 softmax
    epilogue (exact: w1 = 1/(1+e^(v2-v1)), w2 = e*w1).
  - Outputs are written in a HW-friendly [128, tiles*2] layout and
    de-interleaved on the host.
"""

import sys

sys.path.insert(0, "/opt/trn_rl_repo")

import numpy as np
import ml_dtypes

import concourse.bass as bass
import concourse.mybir as mybir
import concourse.tile as tile
from concourse import bacc
from concourse.bass_utils import run_bass_kernel_spmd
from concourse.masks import make_identity

F32 = mybir.dt.float32
BF16 = mybir.dt.bfloat16
I32 = mybir.dt.int32
U32 = mybir.dt.uint32
AF = mybir.ActivationFunctionType
OP = mybir.AluOpType
AX = mybir.AxisListType

N_CORES = 8
N, D, E, K = 32768, 1024, 64, 2
NT = N // N_CORES            # 4096 tokens per core
H = 256
G_TOK = 512                  # tokens per matmul group
N_GROUPS = NT // G_TOK       # 8
TILES = NT // 128            # 32 token tiles of 128
DC = D // 128                # 8 contraction chunks

BF = ml_dtypes.bfloat16

# ---- weight blobs (partition-major [128, *] f32) ----
# blob A: needed at the start of the TTHA chain
_WA_SEGS = [("catT", 4), ("cat_hw", 1), ("wc", 4 * 256), ("wh", 256)]
# blob B: needed a few microseconds later
_WB_SEGS = [
    ("wqkv", 2 * 768), ("wo", 2 * 256), ("wf", 2 * 256),
    ("wou1", 2 * 192), ("wo2", 64), ("wu2", 64),
]


def _offsets(segs):
    out, off = {}, 0
    for nm, w in segs:
        out[nm] = off
        off += w
    return out, off


_WA_OFF, WACOLS = _offsets(_WA_SEGS)
_WB_OFF, WBCOLS = _offsets(_WB_SEGS)

_B1_SEGS = [
    ("embb", 512), ("ln1g", 512), ("ln1b", 512), ("bf", 256), ("gf", 256),
    ("bef", 256), ("bo1", 128), ("bo2", 64), ("bu1", 64), ("bu2", 64),
    ("bgate", 64), ("bqkv", 768), ("bo", 256), ("bou1", 192),
]
_B1_OFF = {}
_off = 0
for _nm, _w in _B1_SEGS:
    _B1_OFF[_nm] = _off
    _off += _w
B1COLS = _off


def _act(nc, out, in_, func, scale=1.0, bias=0.0):
    nc.scalar.activation(out, in_, func, scale=scale, bias=bias)


def _layer_norm_gelu(nc, sb, ps_ap, g_ap, b_ap, out_ap, groups, width, tag):
    """out = gelu(LN(ps) * g + b) on one partition; LN per group of `width`.

    Chain uses ONLY ACT (reductions via accum_out, per-group scalars as
    bias/scale APs) + GpSimd scalar fixups + the gelu — the Vector engine
    stays free for main-loop throughput work.
    """
    AFT = AF
    scratch = sb.tile([1, groups, width], F32, tag=f"{tag}_scr")
    xc = sb.tile([1, groups, width], F32, tag=f"{tag}_xc")
    xn = sb.tile([1, groups, width], F32, tag=f"{tag}_xn")
    mu = sb.tile([1, groups, 1], F32, tag=f"{tag}_mu")
    ssq = sb.tile([1, groups, 1], F32, tag=f"{tag}_ssq")
    g3 = g_ap.rearrange("o (g w) -> o g w", g=groups)
    b3 = b_ap.rearrange("o (g w) -> o g w", g=groups)
    o3 = out_ap.rearrange("o (g w) -> o g w", g=groups)
    x3 = ps_ap.rearrange("o (g w) -> o g w", g=groups)

    for gi in range(groups):
        nc.scalar.activation(scratch[:, gi, :], x3[:, gi, :], AFT.Copy,
                             accum_out=mu[:, gi, :])
    # neg_mu = -sum/width
    nc.gpsimd.tensor_scalar(mu[:], mu[:], -1.0 / width, None, op0=OP.mult)
    for gi in range(groups):
        nc.scalar.activation(xc[:, gi, :], x3[:, gi, :], AFT.Identity,
                             bias=mu[:, gi, :])
        nc.scalar.activation(scratch[:, gi, :], xc[:, gi, :], AFT.Square,
                             accum_out=ssq[:, gi, :])
    # inv_std = sqrt(1/(ssq/width + eps))
    nc.gpsimd.tensor_scalar(ssq[:], ssq[:], 1.0 / width, 1e-5, op0=OP.mult,
                            op1=OP.add)
    nc.vector.reciprocal(ssq[:, :, 0], ssq[:, :, 0])
    nc.scalar.activation(ssq[:, :, 0], ssq[:, :, 0], AFT.Sqrt)
    for gi in range(groups):
        nc.scalar.activation(xn[:, gi, :], xc[:, gi, :], AFT.Identity,
                             scale=ssq[:, gi, :])
    nc.gpsimd.tensor_tensor(xn[:], xn[:], g3, op=OP.mult)
    nc.gpsimd.tensor_tensor(xn[:], xn[:], b3, op=OP.add)
    nc.scalar.activation(o3, xn[:], AFT.Gelu)


def build():
    nc = bacc.Bacc(target_bir_lowering=False)

    xz = nc.dram_tensor("xz", [2 * D, NT], BF16, kind="ExternalInput")
    wgz = nc.dram_tensor("wgz", [2 * D, 128], BF16, kind="ExternalInput")
    wbloba = nc.dram_tensor("wbloba", [128, WACOLS], F32, kind="ExternalInput")
    wblobb = nc.dram_tensor("wblobb", [128, WBCOLS], F32, kind="ExternalInput")
    bblob1 = nc.dram_tensor("bblob1", [1, B1COLS], F32, kind="ExternalInput")

    out_w = nc.dram_tensor("out_w", [128, TILES * 2], F32, kind="ExternalOutput")
    out_i = nc.dram_tensor("out_i", [128, TILES * 2], I32, kind="ExternalOutput")
    out_rb = nc.dram_tensor("out_rb", [1, E], F32, kind="ExternalOutput")
    out_un = nc.dram_tensor("out_un", [1, E], F32, kind="ExternalOutput")

    with tile.TileContext(nc) as tc:
        with tc.tile_pool(name="const", bufs=1) as cs, \
             tc.tile_pool(name="tt", bufs=1) as ts, \
             tc.tile_pool(name="xs", bufs=8) as xs, \
             tc.tile_pool(name="wk", bufs=3) as wk, \
             tc.tile_pool(name="pmain", bufs=4, space="PSUM") as pmain, \
             tc.tile_pool(name="ptr", bufs=2, space="PSUM") as ptr, \
             tc.tile_pool(name="ptt", bufs=2, space="PSUM") as ptt:

            # ---- DMA plan: the sync engine carries ONLY the 16 x-slab halves
            # (the bandwidth-critical stream); all weights/biases ride the
            # gpsimd SWDGE queue, with the early-TTHA blob first.
            wba = cs.tile([128, WACOLS], F32)
            nc.gpsimd.dma_start(wba[:], wbloba[:])
            b1 = cs.tile([1, B1COLS], F32)
            nc.gpsimd.dma_start(b1[:], bblob1[:])
            wz_s = cs.tile([128, 2 * DC, 128], BF16)
            nc.gpsimd.dma_start(wz_s[:], wgz[:].rearrange("(c p) e -> p c e", p=128))
            wbb = cs.tile([128, WBCOLS], F32)
            nc.gpsimd.dma_start(wbb[:], wblobb[:])

            slabs = []
            for g in range(N_GROUPS):
                xhi_g = xs.tile([128, DC, G_TOK], BF16, tag="xhi")
                xlo_g = xs.tile([128, DC, G_TOK], BF16, tag="xlo")
                nc.sync.dma_start(
                    xhi_g[:], xz[0:D, g * G_TOK:(g + 1) * G_TOK].rearrange(
                        "(c p) t -> p c t", p=128))
                nc.sync.dma_start(
                    xlo_g[:], xz[D:2 * D, g * G_TOK:(g + 1) * G_TOK].rearrange(
                        "(c p) t -> p c t", p=128))
                slabs.append((xhi_g, xlo_g))

            ident = cs.tile([128, 128], F32)
            make_identity(nc, ident[:])

            def wseg(nm, rows=128):
                if nm in _WA_OFF:
                    off = _WA_OFF[nm]
                    w = dict(_WA_SEGS)[nm]
                    return wba[0:rows, off:off + w]
                off = _WB_OFF[nm]
                w = dict(_WB_SEGS)[nm]
                return wbb[0:rows, off:off + w]

            def b1seg(nm, w):
                off = _B1_OFF[nm]
                return b1[:, off:off + w]

            # persistent result buffers
            vbuf8 = cs.tile([128, TILES * 8], F32)
            ibuf8 = cs.tile([128, TILES * 8], U32)
            wbuf = cs.tile([128, TILES, 2], F32)
            obuf = cs.tile([128, TILES, 2], I32)

            # =========================================================
            # TTHA adapter (fp32, replicated per core).
            # Engine plan: PE does matmuls/transposes AND all bias adds
            # (as K=1 ones-row matmuls accumulated into the same PSUM);
            # ACT applies activations/copies directly to PSUM; GpSimd does
            # SBUF elementwise; DVE only the reductions + reciprocal.
            # =========================================================
            _hp = tc.high_priority()
            _hp.__enter__()
            ones1 = ts.tile([3, 128], F32)
            nc.gpsimd.memset(ones1[:], 1.0)

            def bias_mm(ps_ap, rows, nm, width, off=0):
                # ps[r, :] += bias_row  for all r in rows
                nc.tensor.matmul(ps_ap, ones1[0:1, 0:rows],
                                 b1[:, _B1_OFF[nm] + off:_B1_OFF[nm] + off + width],
                                 start=False, stop=True, skip_group_check=True)

            catT = wseg("catT").rearrange("p (c o) -> p c o", c=4)
            wc = wseg("wc").rearrange("p (c n) -> p c n", c=4)
            ps_emb = ptt.tile([1, 2 * H], F32, tag="ptt")
            for c in range(4):
                nc.tensor.matmul(ps_emb[:, 0:H], catT[:, c, :], wc[:, c, :],
                                 start=(c == 0), stop=(c == 3))
            nc.tensor.matmul(ps_emb[:, H:2 * H], wseg("cat_hw", rows=9),
                             wseg("wh", rows=9), start=True, stop=True)
            emb_act = ts.tile([1, 2 * H], F32)
            _layer_norm_gelu(nc, ts, ps_emb[:], b1seg("ln1g", 512), b1seg("ln1b", 512),
                             emb_act[:], groups=2, width=H, tag="ln1")
            # combT [128, 2, 3]
            tc.tile_set_cur_wait(0.028)
            combT = ts.tile([128, 2, 3], F32)
            nc.gpsimd.memset(combT[:], 0.0)
            for c in range(2):
                ps_cb = ptt.tile([128, 2], F32, tag="ptt")
                for j in range(2):
                    nc.tensor.transpose(ps_cb[:, j:j + 1],
                                        emb_act[0:1, (j * H + c * 128):(j * H + c * 128 + 128)],
                                        ident[0:1, 0:1])
                nc.scalar.copy(combT[:, c, 0:2], ps_cb[:])

            # qkv = comb @ Wqkv + bqkv -> [3, 768]
            wqkv = wseg("wqkv").rearrange("p (c n) -> p c n", c=2)
            qkv_s = ts.tile([3, 3 * H], F32)
            for half in range(2):
                ps_q = ptt.tile([3, 384], F32, tag="ptt")
                for c in range(2):
                    nc.tensor.matmul(ps_q[:], combT[:, c, :],
                                     wqkv[:, c, (half * 384):(half * 384 + 384)],
                                     start=(c == 0), stop=False, skip_group_check=True)
                bias_mm(ps_q[:], 3, "bqkv", 384, off=half * 384)
                nc.scalar.copy(qkv_s[:, half * 384:(half + 1) * 384], ps_q[:])

            # qT / kT packed 3 heads per chunk (score scale folded into Exp)
            qT = ts.tile([128, 3, 3], F32)
            kT = ts.tile([128, 3, 3], F32)
            for c in range(3):
                w = 96 if c < 2 else 64
                ps_t = ptt.tile([128, 3], F32, tag="ptt")
                nc.tensor.transpose(ps_t[:w], qkv_s[0:3, c * 96:(c * 96 + w)],
                                    ident[0:3, 0:3])
                nc.scalar.copy(qT[:w, c, :], ps_t[:w])
                ps_t2 = ptt.tile([128, 3], F32, tag="ptt")
                nc.tensor.transpose(ps_t2[:w], qkv_s[0:3, (H + c * 96):(H + c * 96 + w)],
                                    ident[0:3, 0:3])
                nc.scalar.copy(kT[:w, c, :], ps_t2[:w])

            tc.tile_set_cur_wait(0.035)
            ps_sc = ptt.tile([3, 24], F32, tag="ptt")
            for h in range(8):
                c, hh = divmod(h, 3)
                nc.tensor.matmul(ps_sc[:, h * 3:(h + 1) * 3],
                                 qT[hh * 32:(hh + 1) * 32, c, :],
                                 kT[hh * 32:(hh + 1) * 32, c, :],
                                 start=True, stop=True)
            # softmax over j: exp straight off PSUM (scale folded in), heads
            # packed 3-per-96 so THREE transposes cover all 8; row sums on
            # DVE, reciprocal as exp(-ln), normalization on GpSimd.
            s32 = ts.tile([3, 3, 3, 32], F32)     # (chunk, head-in-chunk, j-pad)
            nc.gpsimd.memset(s32[:], 0.0)
            _act(nc, s32[:, 0:2, :, 0:3],
                 ps_sc[:, 0:18].rearrange("p (c s j) -> p c s j", c=2, s=3),
                 AF.Exp, scale=float(1.0 / np.sqrt(32.0)))
            _act(nc, s32[:, 2:3, 0:2, 0:3],
                 ps_sc[:, 18:24].rearrange("p (c s j) -> p c s j", c=1, s=2),
                 AF.Exp, scale=float(1.0 / np.sqrt(32.0)))
            at_s = ts.tile([3, 8, 3], F32)
            ps_at = ptt.tile([3, 24], F32, tag="ptt")
            s32v = s32[:].rearrange("p c s o -> p (c s) o")     # [3, 9, 32]
            for h in range(8):
                c, hh = divmod(h, 3)
                nc.tensor.transpose(ps_at[:, h * 3:(h + 1) * 3],
                                    s32v[0:3, c * 3 + hh, 0:3], ident[0:3, 0:3])
            nc.scalar.copy(at_s[:], ps_at[:].rearrange("p (h j) -> p h j", h=8))
            ps_rs = ptt.tile([3, 8], F32, tag="ptt")
            for h in range(8):
                nc.tensor.matmul(ps_rs[:, h:h + 1], at_s[:, h, :], ones1[0:3, 0:1],
                                 start=True, stop=True)
            rinv = ts.tile([3, 8], F32)
            nc.vector.reciprocal(rinv[:], ps_rs[:])

            ps_ctx = ptt.tile([3, H], F32, tag="ptt")
            for h in range(8):
                nc.tensor.matmul(ps_ctx[:, h * 32:(h + 1) * 32], at_s[:, h, :],
                                 qkv_s[0:3, (2 * H + h * 32):(2 * H + (h + 1) * 32)],
                                 start=True, stop=True)
            ctx_s = ts.tile([3, 8, 32], F32)
            nc.scalar.copy(ctx_s[:], ps_ctx[:].rearrange("p (h d) -> p h d", h=8))
            nc.gpsimd.tensor_tensor(ctx_s[:], ctx_s[:],
                                    rinv[:, :, None].to_broadcast([3, 8, 32]),
                                    op=OP.mult)
            ctx_f = ctx_s[:].rearrange("p h d -> p (h d)")

            ctxT = ts.tile([128, 2, 3], F32)
            for c in range(2):
                ps_ct = ptt.tile([128, 3], F32, tag="ptt")
                nc.tensor.transpose(ps_ct[:], ctx_f[0:3, c * 128:(c + 1) * 128],
                                    ident[0:3, 0:3])
                nc.scalar.copy(ctxT[:, c, :], ps_ct[:])
            wo = wseg("wo").rearrange("p (c n) -> p c n", c=2)
            ps_att = ptt.tile([3, H], F32, tag="ptt")
            for c in range(2):
                nc.tensor.matmul(ps_att[:], ctxT[:, c, :], wo[:, c, :],
                                 start=(c == 0), stop=False, skip_group_check=True)
            bias_mm(ps_att[:], 3, "bo", H)
            att_s = ts.tile([3, H], F32)
            nc.scalar.copy(att_s[:], ps_att[:])

            tc.tile_set_cur_wait(0.044)
            third = ts.tile([3, 1], F32)
            nc.gpsimd.memset(third[:], 1.0 / 3.0)
            ps_mean = ptt.tile([1, H], F32, tag="ptt")
            nc.tensor.matmul(ps_mean[:], third[:], att_s[:], start=True, stop=True)
            mean_s = ts.tile([1, H], F32)
            nc.scalar.copy(mean_s[:], ps_mean[:])

            meanT = ts.tile([128, 2, 1], F32)
            for c in range(2):
                ps_mt = ptt.tile([128, 1], F32, tag="ptt")
                nc.tensor.transpose(ps_mt[:], mean_s[0:1, c * 128:(c + 1) * 128],
                                    ident[0:1, 0:1])
                nc.scalar.copy(meanT[:, c, :], ps_mt[:])
            wf = wseg("wf").rearrange("p (c n) -> p c n", c=2)
            ps_f = ptt.tile([1, H], F32, tag="ptt")
            for c in range(2):
                nc.tensor.matmul(ps_f[:], meanT[:, c, :], wf[:, c, :],
                                 start=(c == 0), stop=False, skip_group_check=True)
            bias_mm(ps_f[:], 1, "bf", H)
            fused = ts.tile([1, H], F32)
            _layer_norm_gelu(nc, ts, ps_f[:], b1seg("gf", H), b1seg("bef", H),
                             fused[:], groups=1, width=H, tag="ln2")

            tc.tile_set_cur_wait(0.050)
            fusedT = ts.tile([128, 2, 1], F32)
            for c in range(2):
                ps_ft = ptt.tile([128, 1], F32, tag="ptt")
                nc.tensor.transpose(ps_ft[:], fused[0:1, c * 128:(c + 1) * 128],
                                    ident[0:1, 0:1])
                nc.scalar.copy(fusedT[:, c, :], ps_ft[:])

            # heads (Wo1|Wu1 merged into one [256,192] matmul)
            wou1 = wseg("wou1").rearrange("p (c n) -> p c n", c=2)
            ps_hu = ptt.tile([1, 192], F32, tag="ptt")
            for c in range(2):
                nc.tensor.matmul(ps_hu[:], fusedT[:, c, :], wou1[:, c, :],
                                 start=(c == 0), stop=False, skip_group_check=True)
            bias_mm(ps_hu[:], 1, "bou1", 192)
            hu = ts.tile([1, 192], F32)
            _act(nc, hu[:], ps_hu[:], AF.Gelu)
            h1 = hu[:, 0:128]
            u1 = hu[:, 128:192]

            h1T = ts.tile([128, 1], F32)
            ps_h1t = ptt.tile([128, 1], F32, tag="ptt")
            nc.tensor.transpose(ps_h1t[:], h1, ident[0:1, 0:1])
            nc.scalar.copy(h1T[:], ps_h1t[:])
            ps_rb = ptt.tile([1, E], F32, tag="ptt")
            nc.tensor.matmul(ps_rb[:], h1T[:], wseg("wo2"), start=True, stop=False,
                             skip_group_check=True)
            bias_mm(ps_rb[:], 1, "bo2", E)
            rb = ts.tile([1, E], F32)
            _act(nc, rb[:], ps_rb[:], AF.Tanh)  # tanh lives in the gelu set
            nc.gpsimd.dma_start(out_rb[:], rb[:])

            u1T = ts.tile([64, 1], F32)
            ps_u1t = ptt.tile([64, 1], F32, tag="ptt")
            nc.tensor.transpose(ps_u1t[:], u1, ident[0:1, 0:1])
            nc.scalar.copy(u1T[:], ps_u1t[:])
            ps_u2 = ptt.tile([1, E], F32, tag="ptt")
            nc.tensor.matmul(ps_u2[:], u1T[:], wseg("wu2", rows=64), start=True,
                             stop=False, skip_group_check=True)
            bias_mm(ps_u2[:], 1, "bu2", E)
            un = ts.tile([1, E], F32)
            _act(nc, un[:], ps_u2[:], AF.Exp)
            nc.gpsimd.tensor_scalar(un[:], un[:], 1.0, None, op0=OP.add)
            _act(nc, un[:], un[:], AF.Ln)
            nc.gpsimd.dma_start(out_un[:], un[:])

            # total per-expert bias broadcast to all 128 partitions via PE
            bt = ts.tile([1, E], F32)
            nc.gpsimd.tensor_tensor(bt[:], rb[:], b1seg("bgate", E), op=OP.add)
            ps_b128 = ptt.tile([128, E], F32, tag="ptt")
            nc.tensor.matmul(ps_b128[:], ones1[0:1, :], bt[:], start=True, stop=True)
            bias128 = cs.tile([128, E], F32)
            nc.scalar.copy(bias128[:], ps_b128[:])
            tc.cur_wait_ts = None
            _hp.__exit__(None, None, None)

            # =========================================================
            # Main gate matmul + top-2.  The PE pipeline (matmuls +
            # transposes) never waits on the TTHA chain: raw logits are
            # transposed into lbuf, and the bias add + top-2 for group g
            # run one group behind (by which time bias128 is ready).
            # =========================================================
            lbuf = cs.tile([128, TILES, E], F32)

            def gated_top2(g):
                tmp4 = wk.tile([128, 4, E], F32, tag="tmp4")
                nc.vector.tensor_tensor(
                    tmp4[:], lbuf[:, g * 4:(g + 1) * 4, :],
                    bias128[:, None, :].to_broadcast([128, 4, E]), op=OP.add)
                for t in range(4):
                    tl = g * 4 + t
                    nc.vector.max(vbuf8[:, tl * 8:(tl + 1) * 8], tmp4[:, t, :])
                    nc.vector.max_index(ibuf8[:, tl * 8:(tl + 1) * 8],
                                        vbuf8[:, tl * 8:(tl + 1) * 8], tmp4[:, t, :])

            for g in range(N_GROUPS):
                xhi_g, xlo_g = slabs[g]
                ps_lg = pmain.tile([128, G_TOK], F32, tag="lg")
                n_mm = 3 * DC
                k = 0
                # hi products first (only need the hi half of the slab)
                for c in range(DC):
                    nc.tensor.matmul(ps_lg[:], wz_s[:, c, :], xhi_g[:, c, :],
                                     start=(k == 0), stop=False)
                    k += 1
                for c in range(DC):
                    nc.tensor.matmul(ps_lg[:], wz_s[:, c, :], xlo_g[:, c, :],
                                     start=False, stop=False)
                    k += 1
                    nc.tensor.matmul(ps_lg[:], wz_s[:, DC + c, :], xhi_g[:, c, :],
                                     start=False, stop=(k == n_mm - 1))
                    k += 1

                # raw (unbiased) logits to SBUF (DVE; ACT stays pure-chain)
                adj = wk.tile([E, G_TOK], F32, tag="adj")
                nc.vector.tensor_copy(adj[:], ps_lg[0:E, :])

                ps_tr = ptr.tile([128, 4, E], F32, tag="tr")
                for t in range(4):
                    nc.tensor.transpose(ps_tr[:, t, :], adj[:, t * 128:(t + 1) * 128],
                                        ident[0:E, 0:E])
                nc.vector.tensor_copy(lbuf[:, g * 4:(g + 1) * 4, :], ps_tr[:])

                if g >= 1:
                    gated_top2(g - 1)
            gated_top2(N_GROUPS - 1)

            # ---- top-2 softmax epilogue ----
            v3 = vbuf8[:].rearrange("p (t k) -> p t k", k=8)
            i3 = ibuf8[:].rearrange("p (t k) -> p t k", k=8)
            d_t = wk.tile([128, TILES, 1], F32, tag="dt")
            nc.vector.tensor_tensor(d_t[:], v3[:, :, 1:2], v3[:, :, 0:1], op=OP.subtract)
            _act(nc, d_t[:], d_t[:], AF.Exp)
            s_t = wk.tile([128, TILES, 1], F32, tag="st")
            nc.vector.tensor_scalar(s_t[:], d_t[:], 1.0, None, op0=OP.add)
            nc.vector.reciprocal(wbuf[:, :, 0:1], s_t[:])
            nc.vector.tensor_tensor(wbuf[:, :, 1:2], d_t[:], wbuf[:, :, 0:1], op=OP.mult)
            nc.vector.tensor_copy(obuf[:, :, 0:1], i3[:, :, 0:1])
            nc.vector.tensor_copy(obuf[:, :, 1:2], i3[:, :, 1:2])
            nc.sync.dma_start(out_w[:], wbuf[:])
            nc.sync.dma_start(out_i[:], obuf[:])

    nc.finalize()
    return nc


_CACHE = {}


def _get_nc():
    if "nc" not in _CACHE:
        _CACHE["nc"] = build()
    return _CACHE["nc"]


def kernel(**inputs):
    f32 = np.float32
    g = {k: np.asarray(v, f32) for k, v in inputs.items()}
    x = g["x"]

    wg = g["W_gate"]
    wghi = wg.astype(BF)
    wglo = (wg - wghi.astype(f32)).astype(BF)
    wgz = np.zeros((2 * D, 128), BF)
    wgz[:, 0:E] = np.concatenate([wghi, wglo], axis=0)

    wbloba = np.zeros((128, WACOLS), f32)
    wblobb = np.zeros((128, WBCOLS), f32)

    def put_w(nm, arr):
        blob, off = ((wbloba, _WA_OFF[nm]) if nm in _WA_OFF
                     else (wblobb, _WB_OFF[nm]))
        arr = np.asarray(arr, f32)
        blob[:arr.shape[0], off:off + arr.shape[1]] = arr

    cost = g["cost_features"][0]
    catT_arr = np.zeros((128, 4), f32)
    catT_arr[:, 0:3] = cost.reshape(3, 128).T
    catT_arr[0, 3] = 1.0
    put_w("catT", catT_arr)
    cat_hw = np.zeros((9, 1), f32)
    cat_hw[0:8, 0] = g["hardware_features"][0]
    cat_hw[8, 0] = 1.0
    put_w("cat_hw", cat_hw)
    wc_arr = np.zeros((128, 4, 256), f32)
    wc_arr[:, 0:3, :] = g["Wc"].reshape(3, 128, 256).transpose(1, 0, 2)
    wc_arr[0, 3, :] = g["bc"]
    put_w("wc", wc_arr.reshape(128, 4 * 256))
    wh_arr = np.zeros((9, 256), f32)
    wh_arr[0:8] = g["Wh"]
    wh_arr[8] = g["bh"]
    put_w("wh", wh_arr)
    put_w("wqkv", g["Wqkv"].reshape(2, 128, 768).transpose(1, 0, 2).reshape(128, 2 * 768))
    put_w("wo", g["Wo"].reshape(2, 128, 256).transpose(1, 0, 2).reshape(128, 2 * 256))
    put_w("wf", g["Wf"].reshape(2, 128, 256).transpose(1, 0, 2).reshape(128, 2 * 256))
    wou1 = np.concatenate([g["Wo1"], g["Wu1"]], axis=1)
    put_w("wou1", wou1.reshape(2, 128, 192).transpose(1, 0, 2).reshape(128, 2 * 192))
    put_w("wo2", g["Wo2"])
    put_w("wu2", g["Wu2"])

    bblob1 = np.zeros((1, B1COLS), f32)

    def put_b(nm, arr):
        off = _B1_OFF[nm]
        arr = np.asarray(arr, f32).reshape(-1)
        bblob1[0, off:off + arr.size] = arr

    put_b("embb", np.concatenate([g["bc"], g["bh"]]))
    put_b("ln1g", np.concatenate([g["gc"], g["gh"]]))
    put_b("ln1b", np.concatenate([g["bec"], g["beh"]]))
    put_b("bf", g["bf"]); put_b("gf", g["gf"]); put_b("bef", g["bef"])
    put_b("bo1", g["bo1"]); put_b("bo2", g["bo2"])
    put_b("bu1", g["bu1"]); put_b("bu2", g["bu2"])
    put_b("bou1", np.concatenate([g["bo1"], g["bu1"]]))
    put_b("bgate", g["b_gate"])

    put_b("bqkv", g["bqkv"])
    put_b("bo", g["bo"])

    shared = dict(wgz=wgz, wbloba=wbloba, wblobb=wblobb, bblob1=bblob1)

    in_maps = []
    for c in range(N_CORES):
        xs = np.ascontiguousarray(x[c * NT:(c + 1) * NT].T)
        xhi = xs.astype(BF)
        xlo = (xs - xhi.astype(f32)).astype(BF)
        in_maps.append(dict(shared, xz=np.concatenate([xhi, xlo], axis=0)))

    nc = _get_nc()
    res = run_bass_kernel_spmd(nc, in_maps, core_ids=list(range(N_CORES)))

    weights = np.empty((N, K), f32)
    top_idx = np.empty((N, K), np.int32)
    for c in range(N_CORES):
        r = res.results[c]
        weights[c * NT:(c + 1) * NT] = (
            r["out_w"].reshape(128, TILES, 2).transpose(1, 0, 2).reshape(NT, 2))
        top_idx[c * NT:(c + 1) * NT] = (
            r["out_i"].reshape(128, TILES, 2).transpose(1, 0, 2).reshape(NT, 2))
    rb = res.results[0]["out_rb"].reshape(1, E).astype(f32)
    un = res.results[0]["out_un"].reshape(1, E).astype(f32)
    return weights, top_idx, rb, un
